# revision 14
# baseline (speedup 1.0000x reference)
"""Trainium2 Bass kernel for LogWignerCrystalSlaterFixedCYJastrow.

Per walker (1024 walkers, 64 electrons, box L=20):
    out = logdet(Phi_up) + logdet(Phi_dn) + jastrow
Phi_s are 32x32 Gaussian-orbital Slater matrices (27 periodic images
collapsed to separable per-axis 3-image sums), jastrow is a Coulomb-Yukawa
pair sum with minimum-image wrapping.

Data parallel over 8 NeuronCores, 128 walkers/core, one walker per SBUF
partition.  v2 optimizations over the first working kernel:
  * Khatri-Rao orbital build: centers take only 2/4/4 distinct values per
    axis, so per-axis factors are [32 x M] (M=2 or 4) instead of [32 x 32];
    Phi is expanded with two broadcast multiplies.  ~8x less phi chain work.
  * Circulant jastrow: pairs (i, i+d mod 64) for d=1..32 cover each
    unordered pair once (d=32 plane twice -> half weight), so all pair work
    is [32 x 64] instead of [64 x 64] and there is no diagonal to mask.
  * GE pivot machinery: fixed tie-break weights + exact self-annihilation
    (divide gives m[i*] = piv/piv = 1.0 exactly, so used rows become exact
    zeros and can never be picked again) -> no mask update op.  Pivot row
    extraction in one pass via copy_predicated with a stride-0 output axis
    (falls back to mult+reduce).
"""

import os
import sys
import numpy as np
from contextlib import ExitStack

for _p in ("/opt/trn_rl_repo", "/opt/pypackages"):
    if _p not in sys.path:
        sys.path.append(_p)

import concourse.bass as bass
import concourse.bacc as bacc
import concourse.mybir as mybir
import concourse.tile as tile
from concourse.bass import AP
from concourse.bass_utils import run_bass_kernel_spmd

P = 128          # partitions = walkers per core
NCORES = 8
B = 1024
N = 64           # electrons per walker
NS = 32          # electrons / orbitals per spin
ND = 32          # circulant difference planes (d = 1..32)
L = 20.0
F32 = mybir.dt.float32
AF = mybir.ActivationFunctionType
OP = mybir.AluOpType
AX = mybir.AxisListType

USE_CP = False   # pivot-row extraction via copy_predicated (1 pass)
USE_DIV = True   # multipliers via tensor_tensor divide (exact self-annihilation)


def _jastrow_consts():
    dens = np.float32(N / L ** 3)
    A = np.float32(1.0) / np.sqrt(np.float32(4 * np.pi) * dens, dtype=np.float32)
    Fs = np.sqrt(np.float32(2.0) * A, dtype=np.float32)
    Fd = np.sqrt(A, dtype=np.float32)
    return float(A), float(Fs), float(Fd)


def _ap(t, extra_off, axes):
    """Free-dim view of tile t: axes = [(stride, size), ...] in elements."""
    return AP(t.tensor, t.offset + extra_off, [list(t.ap[0])] + [list(a) for a in axes])


def _build(alpha: float) -> bass.Bass:
    nc = bacc.Bacc()
    xsh = nc.declare_dram_parameter("xsh", [P, 3, N], F32, isOutput=False)
    cst = nc.declare_dram_parameter("cst", [P, 2, 10], F32, isOutput=False)
    wcs = nc.declare_dram_parameter("wcs", [P, 2, NS], F32, isOutput=False)
    fmi = nc.declare_dram_parameter("fmi", [P, ND, N], F32, isOutput=False)
    outp = nc.declare_dram_parameter("out", [P, 1], F32, isOutput=True)

    aL2 = float(alpha * L * L)
    s2aL = float(2.0 * alpha * L)
    Aj, Fs, Fd = _jastrow_consts()
    WMIN = float(1.0 - (1.0 - 1e-5) ** 2)   # lower clamp of w = 1 - x^2

    with ExitStack() as ctx:
        tc = ctx.enter_context(tile.TileContext(nc))
        pool = ctx.enter_context(tc.tile_pool(name="main", bufs=1))

        # ---- loads ----
        xdup = pool.tile([P, 3, 2 * N], F32, tag="xdup")
        nc.default_dma_engine.dma_start(xdup[:, :, 0:N], xsh[:])
        nc.default_dma_engine.dma_start(xdup[:, :, N:2 * N], xsh[:])
        ce = pool.tile([P, 2, 10], F32, tag="ce")
        nc.default_dma_engine.dma_start(ce, cst[:])
        Wt = pool.tile([P, 2, NS], F32, tag="Wt")
        nc.default_dma_engine.dma_start(Wt, wcs[:])
        fm = pool.tile([P, ND, N], F32, tag="fm")
        nc.default_dma_engine.dma_start(fm, fmi[:])

        half = float(L / 2)
        biasc = pool.tile([P, 7], F32, tag="biasc")
        nc.gpsimd.memset(biasc[:, 0:1], -aL2)        # Exp image bias
        nc.gpsimd.memset(biasc[:, 1:2], -half)       # Abs bias
        nc.gpsimd.memset(biasc[:, 2:3], half)        # Square bias
        nc.gpsimd.memset(biasc[:, 3:4], 1e-37)       # Ln guard
        nc.gpsimd.memset(biasc[:, 4:5], WMIN)        # Ln w bias
        nc.gpsimd.memset(biasc[:, 5:6], 1.0 - WMIN)  # Relu bias
        nc.gpsimd.memset(biasc[:, 6:7], 1.0)         # decay Exp bias

        # ---- tiles ----
        Abuf = pool.tile([P, 2, NS, NS], F32, tag="Abuf")    # A[p,s,j,i] col-major
        prow = pool.tile([P, 2, NS, NS], F32, tag="prow")    # extracted pivot rows
        scr = pool.tile([P, 2, NS, NS], F32, tag="scr")
        t1 = pool.tile([P, 2, NS, 16], F32, tag="t1")        # Fy*Fz (i, jyz)
        c2b = pool.tile([P, 2, NS], F32, tag="c2b")
        c2m = pool.tile([P, 2, NS], F32, tag="c2m")
        Mball = pool.tile([P, 2, NS], F32, tag="Mball")
        indb = pool.tile([P, 2, NS], mybir.dt.uint8, tag="indb")
        mm = pool.tile([P, 2, NS], F32, tag="mm")
        rpv = pool.tile([P, 2, 1], F32, tag="rpv")
        onec = pool.tile([P, 1], F32, tag="onec")
        nc.gpsimd.memset(onec, 1.0)
        jsum = pool.tile([P, 1], F32, tag="jsum")

        jacc = pool.tile([P, ND, N], F32, tag="jacc")
        j1 = pool.tile([P, ND, N], F32, tag="j1")
        j2 = pool.tile([P, ND, N], F32, tag="j2")
        j3 = pool.tile([P, ND, N], F32, tag="j3")
        j4 = pool.tile([P, ND, N], F32, tag="j4")
        j5 = pool.tile([P, ND, N], F32, tag="j5")

        # per-axis orbital factors (Khatri-Rao): Fx [P,2,32,2], Fy/Fz [P,2,32,4]
        Fx = pool.tile([P, 2, NS, 2], F32, tag="Fx")
        Fy = pool.tile([P, 2, NS, 4], F32, tag="Fy")
        Fz = pool.tile([P, 2, NS, 4], F32, tag="Fz")
        dX = pool.tile([P, 2, NS, 2], F32, tag="dX")
        dY = pool.tile([P, 2, NS, 4], F32, tag="dY")
        dZ = pool.tile([P, 2, NS, 4], F32, tag="dZ")
        sX = pool.tile([P, 2, NS, 2], F32, tag="sX")
        sY = pool.tile([P, 2, NS, 4], F32, tag="sY")
        sZ = pool.tile([P, 2, NS, 4], F32, tag="sZ")
        pX = pool.tile([P, 2, NS, 2], F32, tag="pX")
        pY = pool.tile([P, 2, NS, 4], F32, tag="pY")
        pZ = pool.tile([P, 2, NS, 4], F32, tag="pZ")

        # =========================================================
        # jastrow circulant diffs first (DVE), so ScalarE wrap chains
        # can run while DVE builds phi / starts GE
        # =========================================================
        # jd[p, d, i] = x[i] - x[(i + d + 1) % 64]
        for c, dst in ((0, j1), (1, j4), (2, j5)):
            xin0 = _ap(xdup, c * 2 * N, [(0, ND), (1, N)])
            xin1 = _ap(xdup, c * 2 * N + 1, [(1, ND), (1, N)])
            nc.vector.tensor_tensor(dst, xin0, xin1, OP.subtract)

        # =========================================================
        # phi factors (Khatri-Rao, per axis): d, Square, e0, p+, p-
        # =========================================================
        axdefs = (
            (0, dX, sX, pX, Fx, 0, 2),   # (coord, d, sq, pp, F, center-offset, M)
            (1, dY, sY, pY, Fy, 2, 4),
            (2, dZ, sZ, pZ, Fz, 6, 4),
        )
        for c, d, sq, pp, F, coff, M in axdefs:
            xin = _ap(xdup, c * 2 * N, [(NS, 2), (1, NS), (0, M)])
            cin = _ap(ce, coff, [(10, 2), (0, NS), (1, M)])
            nc.vector.tensor_tensor(d, xin, cin, OP.subtract)
        for c, d, sq, pp, F, coff, M in axdefs:
            nc.scalar.activation(sq, d, AF.Square)
            nc.scalar.activation(pp, d, AF.Exp, bias=biasc[:, 0:1], scale=-s2aL)
            nc.scalar.activation(F, d, AF.Exp, bias=biasc[:, 0:1], scale=s2aL)
            nc.scalar.activation(sq, sq, AF.Exp, scale=-alpha)        # e0 (in place)
            nc.vector.tensor_tensor(pp, pp, F, OP.add)                # q = p+ + p-
            nc.vector.scalar_tensor_tensor(F, pp, 1.0, sq, OP.add, OP.mult)

        # ---- expansion: Abuf[p,s,j,i] = Fx[i,jx]*Fy[i,jy]*Fz[i,jz],
        #      j = jx*16 + jy*4 + jz  (per spin: ISA allows max 3 free axes) ----
        for s in range(2):
            nc.vector.tensor_tensor(
                _ap(t1, s * 512, [(16, NS), (4, 4), (1, 4)]),     # (i, jy, jz)
                _ap(Fy, s * NS * 4, [(4, NS), (1, 4), (0, 4)]),
                _ap(Fz, s * NS * 4, [(4, NS), (0, 4), (1, 4)]),
                OP.mult,
            )
        for s in range(2):
            nc.vector.tensor_tensor(
                _ap(Abuf, s * NS * NS, [(NS, NS), (16, 2), (1, 16)]),   # (i, jx, jyz)
                _ap(t1, s * 512, [(16, NS), (0, 2), (1, 16)]),
                _ap(Fx, s * NS * 2, [(2, NS), (1, 2), (0, 16)]),
                OP.mult,
            )

        # =========================================================
        # jastrow ScalarE wrap chains (queued after phi ScalarE work)
        # =========================================================
        for jsrc, tmp, dst in ((j1, j2, jacc), (j4, j2, j3), (j5, j2, j2)):
            nc.scalar.activation(tmp, jsrc, AF.Abs)
            nc.scalar.activation(jsrc, tmp, AF.Abs, bias=biasc[:, 1:2])
            nc.scalar.activation(dst, jsrc, AF.Square, bias=biasc[:, 2:3], scale=-1.0)

        # =========================================================
        # batched GE with virtual partial pivoting
        # =========================================================
        def jslot(k):
            """DVE/ScalarE jastrow ops interleaved into the GE stream."""
            if k == 4:
                nc.vector.tensor_tensor(jacc, jacc, j3, OP.add)
            elif k == 5:
                nc.vector.tensor_tensor(jacc, jacc, j2, OP.add)      # jacc = r2
            elif k == 6:
                nc.scalar.activation(j2, jacc, AF.Ln, bias=biasc[:, 3:4])     # ln r2
                nc.scalar.activation(j3, jacc, AF.Relu,
                                     bias=biasc[:, 5:6], scale=-0.01)    # w - WMIN
                nc.scalar.activation(j5, j2, AF.Exp, scale=-0.5)      # q = 1/r
                nc.scalar.activation(jacc, j2, AF.Exp, scale=0.5)     # r
            elif k == 8:
                nc.vector.tensor_tensor(j1, jacc, fm, OP.mult)        # -r/F
                nc.scalar.activation(j2, j1, AF.Exp)                  # e
                nc.scalar.activation(j1, j3, AF.Ln, bias=biasc[:, 4:5])        # ln w
                nc.scalar.activation(j3, j1, AF.Exp, scale=-1.0)      # 1/w
                nc.scalar.activation(j1, j3, AF.Exp, bias=biasc[:, 6:7], scale=-1.0)  # decay
            elif k == 10:
                # om = 1 - e  (d=32 plane gets half weight)
                nc.scalar.activation(j3[:, 0:ND - 1, :], j2[:, 0:ND - 1, :],
                                     AF.Copy, bias=1.0, scale=-1.0)
                nc.scalar.activation(j3[:, ND - 1, :], j2[:, ND - 1, :],
                                     AF.Copy, bias=0.5, scale=-0.5)
            elif k == 12:
                nc.vector.tensor_tensor(j2, j3, j1, OP.mult)          # om*decay
            elif k == 14:
                nc.vector.tensor_tensor(j1, j2, j5, OP.mult)          # *q
            elif k == 16:
                nc.scalar.activation(j2, j1, AF.Copy, scale=-Aj, accum_out=jsum)

        for k in range(NS):
            T = NS - k
            colk = _ap(Abuf, k, [(NS * NS, 2), (NS, NS)])   # column k (strided)
            jslot(k)
            # ---- pivot search ----
            nc.vector.tensor_tensor(c2b, colk, colk, OP.mult)
            nc.vector.tensor_tensor(c2m, c2b, Wt, OP.mult)
            nc.vector.reduce_max(_ap(Mball, k, [(NS, 2), (1, 1)]), c2m, axis=AX.X)
            nc.vector.tensor_tensor(
                indb, c2m, _ap(Mball, k, [(NS, 2), (0, NS)]), OP.is_equal
            )
            # ---- pivot row extraction into prow[:, :, k, 0:T] ----
            if USE_CP:
                # out j-run contiguous; stride-0 i axis outermost; data reads
                # contiguous (row-major rows); exactly one i per (p,s) has mask=1
                nc.vector.copy_predicated(
                    _ap(prow, k * NS, [(NS * NS, 2), (0, NS), (1, T)]),
                    _ap(indb, 0, [(NS, 2), (1, NS), (0, T)]),
                    _ap(Abuf, k, [(NS * NS, 2), (NS, NS), (1, T)]),
                )
            else:
                nc.vector.tensor_tensor(
                    _ap(scr, 0, [(NS * NS, 2), (NS, NS), (1, T)]),
                    _ap(Abuf, k, [(NS * NS, 2), (NS, NS), (1, T)]),
                    _ap(indb, 0, [(NS, 2), (1, NS), (0, T)]),
                    OP.mult,
                )
                nc.vector.reduce_sum(
                    _ap(prow, k * NS, [(NS * NS, 2), (1, T)]),
                    _ap(scr, 0, [(NS * NS, 2), (1, T), (NS, NS)]),
                    axis=AX.X,
                )
            if k == NS - 1:
                break
            # ---- multipliers: m = colk / piv, with m[i*] forced to exactly
            # 1.0 so the pivot row self-annihilates to exact zeros (used rows
            # then always lose the max search; no mask bookkeeping needed) ----
            nc.vector.reciprocal(rpv, _ap(prow, k * NS, [(NS * NS, 2), (1, 1)]))
            nc.vector.tensor_tensor(
                mm, colk, rpv.broadcast_to([P, 2, NS]), OP.mult
            )
            nc.vector.copy_predicated(mm, indb, _ap(onec, 0, [(0, 2), (0, NS)]))
            # ---- rank-1 update of trailing columns (row-major) ----
            nc.vector.tensor_tensor(
                _ap(scr, 0, [(NS * NS, 2), (NS, NS), (1, T - 1)]),
                _ap(mm, 0, [(NS, 2), (1, NS), (0, T - 1)]),
                _ap(prow, k * NS + 1, [(NS * NS, 2), (0, NS), (1, T - 1)]),
                OP.mult,
            )
            nc.vector.tensor_tensor(
                _ap(Abuf, k + 1, [(NS * NS, 2), (NS, NS), (1, T - 1)]),
                _ap(Abuf, k + 1, [(NS * NS, 2), (NS, NS), (1, T - 1)]),
                _ap(scr, 0, [(NS * NS, 2), (NS, NS), (1, T - 1)]),
                OP.subtract,
            )

        # =========================================================
        # logdet tail + combine
        # =========================================================
        pivs = _ap(prow, 0, [(NS * NS, 2), (NS, NS)])   # prow[:, :, k, 0] over k
        labs = pool.tile([P, 2, NS], F32, tag="labs")
        lgb = pool.tile([P, 2, NS], F32, tag="lgb")
        nc.scalar.activation(labs, pivs, AF.Abs)
        nc.scalar.activation(lgb, labs, AF.Ln, bias=biasc[:, 3:4])
        ld2 = pool.tile([P, 2], F32, tag="ld2")
        nc.vector.reduce_sum(ld2, lgb, axis=AX.X)
        ld1 = pool.tile([P, 1], F32, tag="ld1")
        nc.vector.reduce_sum(ld1, ld2, axis=AX.X)
        ob = pool.tile([P, 1], F32, tag="ob")
        nc.vector.tensor_tensor(ob, ld1, jsum, OP.add)
        nc.default_dma_engine.dma_start(outp[:], ob)

    nc.finalize()
    return nc


_CACHE = {}


def _get_built(alpha: float):
    key = round(alpha, 9)
    if key not in _CACHE:
        _CACHE[key] = _build(alpha)
    return _CACHE[key]


def _make_inputs(walkerRs: np.ndarray):
    n = 4
    a = L / n
    coords = np.linspace(0.0, L - a, n).astype(np.float32)
    cen = np.zeros((2, 10), np.float32)
    for s, shift in ((0, 0.0), (1, a / 2)):
        cen[s, 0:2] = coords[:2] + shift
        cen[s, 2:6] = coords[:4] + shift
        cen[s, 6:10] = coords[:4] + shift
    cstv = np.ascontiguousarray(np.broadcast_to(cen[None], (P, 2, 10))).astype(np.float32)

    w = (1.0 + np.arange(NS) * 2.0 ** -20).astype(np.float32)
    wcsv = np.ascontiguousarray(np.broadcast_to(w[None, None, :], (P, 2, NS))).astype(np.float32)

    _, Fs, Fd = _jastrow_consts()
    ii = np.arange(N)
    ddv = np.arange(1, ND + 1)
    same = ((ii[None, :] < NS) == (((ii[None, :] + ddv[:, None]) % N) < NS))
    fmv = np.where(same, -1.0 / Fs, -1.0 / Fd).astype(np.float32)
    fmv = np.ascontiguousarray(np.broadcast_to(fmv[None], (P, ND, N))).astype(np.float32)

    in_maps = []
    for c in range(NCORES):
        sh = walkerRs[c * P:(c + 1) * P]          # (P, N, 3)
        xshv = np.ascontiguousarray(sh.transpose(0, 2, 1)).astype(np.float32)
        in_maps.append({"xsh": xshv, "cst": cstv, "wcs": wcsv, "fmi": fmv})
    return in_maps


def kernel(walkerRs: np.ndarray, log_alpha: np.ndarray, _trace=False):
    walkerRs = np.asarray(walkerRs, dtype=np.float32)
    la = float(np.asarray(log_alpha))
    alpha = float(np.clip(np.exp(la), 55.0 / L ** 2, 300.0 / L ** 2))
    nc = _get_built(alpha)
    in_maps = _make_inputs(walkerRs)
    res = None
    for attempt in range(3):
        try:
            res = run_bass_kernel_spmd(nc, in_maps, list(range(NCORES)),
                                       trace=_trace)
            break
        except Exception:
            # transient NRT "device unrecoverable" after a prior bad run
            if attempt == 2:
                raise
            import time as _time
            _time.sleep(15)
    out = np.concatenate([res.results[i]["out"][:, 0] for i in range(NCORES)])
    if _trace:
        return out.astype(np.float32), res
    return out.astype(np.float32)


# revision 17
# speedup vs baseline: 1.0841x; 1.0841x over previous
"""Trainium2 Bass kernel for LogWignerCrystalSlaterFixedCYJastrow.

Per walker (1024 walkers, 64 electrons, box L=20):
    out = logdet(Phi_up) + logdet(Phi_dn) + jastrow
Phi_s are 32x32 Gaussian-orbital Slater matrices (27 periodic images
collapsed to separable per-axis 3-image sums), jastrow is a Coulomb-Yukawa
pair sum with minimum-image wrapping.

Data parallel over 8 NeuronCores, 128 walkers/core, one walker per SBUF
partition.  v2 optimizations over the first working kernel:
  * Khatri-Rao orbital build: centers take only 2/4/4 distinct values per
    axis, so per-axis factors are [32 x M] (M=2 or 4) instead of [32 x 32];
    Phi is expanded with two broadcast multiplies.  ~8x less phi chain work.
  * Circulant jastrow: pairs (i, i+d mod 64) for d=1..32 cover each
    unordered pair once (d=32 plane twice -> half weight), so all pair work
    is [32 x 64] instead of [64 x 64] and there is no diagonal to mask.
  * GE pivot machinery: fixed tie-break weights + exact self-annihilation
    (divide gives m[i*] = piv/piv = 1.0 exactly, so used rows become exact
    zeros and can never be picked again) -> no mask update op.  Pivot row
    extraction in one pass via copy_predicated with a stride-0 output axis
    (falls back to mult+reduce).
"""

import os
import sys
import numpy as np
from contextlib import ExitStack

for _p in ("/opt/trn_rl_repo", "/opt/pypackages"):
    if _p not in sys.path:
        sys.path.append(_p)

import concourse.bass as bass
import concourse.bacc as bacc
import concourse.mybir as mybir
import concourse.tile as tile
from concourse.bass import AP
from concourse.bass_utils import run_bass_kernel_spmd

P = 128          # partitions = walkers per core
NCORES = 8
B = 1024
N = 64           # electrons per walker
NS = 32          # electrons / orbitals per spin
ND = 32          # circulant difference planes (d = 1..32)
L = 20.0
F32 = mybir.dt.float32
AF = mybir.ActivationFunctionType
OP = mybir.AluOpType
AX = mybir.AxisListType

USE_CP = False   # pivot-row extraction via copy_predicated (1 pass)
USE_DIV = True   # multipliers via tensor_tensor divide (exact self-annihilation)


def _jastrow_consts():
    dens = np.float32(N / L ** 3)
    A = np.float32(1.0) / np.sqrt(np.float32(4 * np.pi) * dens, dtype=np.float32)
    Fs = np.sqrt(np.float32(2.0) * A, dtype=np.float32)
    Fd = np.sqrt(A, dtype=np.float32)
    return float(A), float(Fs), float(Fd)


def _ap(t, extra_off, axes):
    """Free-dim view of tile t: axes = [(stride, size), ...] in elements."""
    return AP(t.tensor, t.offset + extra_off, [list(t.ap[0])] + [list(a) for a in axes])


def _build(alpha: float) -> bass.Bass:
    nc = bacc.Bacc()
    xsh = nc.declare_dram_parameter("xsh", [P, 3, N], F32, isOutput=False)
    cst = nc.declare_dram_parameter("cst", [P, 2, 10], F32, isOutput=False)
    wcs = nc.declare_dram_parameter("wcs", [P, 2, NS], F32, isOutput=False)
    fmi = nc.declare_dram_parameter("fmi", [P, ND, N], F32, isOutput=False)
    outp = nc.declare_dram_parameter("out", [P, 1], F32, isOutput=True)

    aL2 = float(alpha * L * L)
    s2aL = float(2.0 * alpha * L)
    Aj, Fs, Fd = _jastrow_consts()
    WMIN = float(1.0 - (1.0 - 1e-5) ** 2)   # lower clamp of w = 1 - x^2

    with ExitStack() as ctx:
        tc = ctx.enter_context(tile.TileContext(nc))
        pool = ctx.enter_context(tc.tile_pool(name="main", bufs=1))

        # ---- loads ----
        xdup = pool.tile([P, 3, 2 * N], F32, tag="xdup")
        nc.default_dma_engine.dma_start(xdup[:, :, 0:N], xsh[:])
        nc.default_dma_engine.dma_start(xdup[:, :, N:2 * N], xsh[:])
        ce = pool.tile([P, 2, 10], F32, tag="ce")
        nc.default_dma_engine.dma_start(ce, cst[:])
        Wt = pool.tile([P, 2, NS], F32, tag="Wt")
        nc.default_dma_engine.dma_start(Wt, wcs[:])
        fm = pool.tile([P, ND, N], F32, tag="fm")
        nc.default_dma_engine.dma_start(fm, fmi[:])

        half = float(L / 2)
        biasc = pool.tile([P, 7], F32, tag="biasc")
        nc.gpsimd.memset(biasc[:, 0:1], -aL2)        # Exp image bias
        nc.gpsimd.memset(biasc[:, 1:2], -half)       # Abs bias
        nc.gpsimd.memset(biasc[:, 2:3], half)        # Square bias
        nc.gpsimd.memset(biasc[:, 3:4], 1e-37)       # Ln guard
        nc.gpsimd.memset(biasc[:, 4:5], WMIN)        # Ln w bias
        nc.gpsimd.memset(biasc[:, 5:6], 1.0 - WMIN)  # Relu bias
        nc.gpsimd.memset(biasc[:, 6:7], 1.0)         # decay Exp bias

        # ---- tiles ----
        Abuf = pool.tile([P, 2, NS, NS], F32, tag="Abuf")    # A[p,s,j,i] col-major
        prow = pool.tile([P, 2, NS, NS], F32, tag="prow")    # extracted pivot rows
        scr = pool.tile([P, 2, NS, NS], F32, tag="scr")
        t1 = pool.tile([P, 2, NS, 16], F32, tag="t1")        # Fy*Fz (jyz, i) packed
        c2b = pool.tile([P, 2, NS], F32, tag="c2b")
        c2m = pool.tile([P, 2, NS], F32, tag="c2m")
        Mb = pool.tile([P, 2], F32, tag="Mb")
        indb = pool.tile([P, 2, NS], mybir.dt.uint8, tag="indb")
        mm = pool.tile([P, 2, NS], F32, tag="mm")
        rpv = pool.tile([P, 2, 1], F32, tag="rpv")
        onec = pool.tile([P, 1], F32, tag="onec")
        nc.gpsimd.memset(onec, 1.0)
        jsum = pool.tile([P, 1], F32, tag="jsum")

        jacc = pool.tile([P, ND, N], F32, tag="jacc")
        j1 = pool.tile([P, ND, N], F32, tag="j1")
        j2 = pool.tile([P, ND, N], F32, tag="j2")
        j3 = pool.tile([P, ND, N], F32, tag="j3")
        j4 = pool.tile([P, ND, N], F32, tag="j4")
        j5 = pool.tile([P, ND, N], F32, tag="j5")

        # per-axis orbital factors (Khatri-Rao): Fx [P,2,32,2], Fy/Fz [P,2,32,4]
        Fx = pool.tile([P, 2, NS, 2], F32, tag="Fx")
        Fy = pool.tile([P, 2, NS, 4], F32, tag="Fy")
        Fz = pool.tile([P, 2, NS, 4], F32, tag="Fz")
        dX = pool.tile([P, 2, NS, 2], F32, tag="dX")
        dY = pool.tile([P, 2, NS, 4], F32, tag="dY")
        dZ = pool.tile([P, 2, NS, 4], F32, tag="dZ")
        sX = pool.tile([P, 2, NS, 2], F32, tag="sX")
        sY = pool.tile([P, 2, NS, 4], F32, tag="sY")
        sZ = pool.tile([P, 2, NS, 4], F32, tag="sZ")
        pX = pool.tile([P, 2, NS, 2], F32, tag="pX")
        pY = pool.tile([P, 2, NS, 4], F32, tag="pY")
        pZ = pool.tile([P, 2, NS, 4], F32, tag="pZ")

        # =========================================================
        # jastrow circulant diffs first (DVE), so ScalarE wrap chains
        # can run while DVE builds phi / starts GE
        # =========================================================
        # jd[p, d, i] = x[i] - x[(i + d + 1) % 64]
        for c, dst in ((0, j1), (1, j4), (2, j5)):
            xin0 = _ap(xdup, c * 2 * N, [(0, ND), (1, N)])
            xin1 = _ap(xdup, c * 2 * N + 1, [(1, ND), (1, N)])
            nc.vector.tensor_tensor(dst, xin0, xin1, OP.subtract)

        # =========================================================
        # phi factors (Khatri-Rao, per axis): d, Square, e0, p+, p-
        # =========================================================
        axdefs = (
            (0, dX, sX, pX, Fx, 0, 2),   # (coord, d, sq, pp, F, center-offset, M)
            (1, dY, sY, pY, Fy, 2, 4),
            (2, dZ, sZ, pZ, Fz, 6, 4),
        )
        for c, d, sq, pp, F, coff, M in axdefs:
            xin = _ap(xdup, c * 2 * N, [(NS, 2), (1, NS), (0, M)])
            cin = _ap(ce, coff, [(10, 2), (0, NS), (1, M)])
            nc.vector.tensor_tensor(d, xin, cin, OP.subtract)
        for c, d, sq, pp, F, coff, M in axdefs:
            nc.scalar.activation(sq, d, AF.Square)
            nc.scalar.activation(pp, d, AF.Exp, bias=biasc[:, 0:1], scale=-s2aL)
            nc.scalar.activation(F, d, AF.Exp, bias=biasc[:, 0:1], scale=s2aL)
            nc.scalar.activation(sq, sq, AF.Exp, scale=-alpha)        # e0 (in place)
            nc.vector.tensor_tensor(pp, pp, F, OP.add)                # q = p+ + p-
            nc.vector.scalar_tensor_tensor(F, pp, 1.0, sq, OP.add, OP.mult)

        # ---- expansion: Abuf[p,s,j,i] = Fx[i,jx]*Fy[i,jy]*Fz[i,jz],
        #      j = jx*16 + jy*4 + jz  (per spin: ISA allows max 3 free axes) ----
        for s in range(2):
            nc.vector.tensor_tensor(
                _ap(t1, s * 512, [(128, 4), (32, 4), (1, NS)]),   # (jy, jz, i)
                _ap(Fy, s * NS * 4, [(1, 4), (0, 4), (4, NS)]),
                _ap(Fz, s * NS * 4, [(0, 4), (1, 4), (4, NS)]),
                OP.mult,
            )
        for s in range(2):
            nc.vector.tensor_tensor(
                _ap(Abuf, s * NS * NS, [(512, 2), (NS, 16), (1, NS)]),  # (jx, jyz, i)
                _ap(t1, s * 512, [(0, 2), (NS, 16), (1, NS)]),
                _ap(Fx, s * NS * 2, [(1, 2), (0, 16), (2, NS)]),
                OP.mult,
            )

        # =========================================================
        # jastrow ScalarE wrap chains (queued after phi ScalarE work)
        # =========================================================
        for jsrc, tmp, dst in ((j1, j2, jacc), (j4, j2, j3), (j5, j2, j2)):
            nc.scalar.activation(tmp, jsrc, AF.Abs)
            nc.scalar.activation(jsrc, tmp, AF.Abs, bias=biasc[:, 1:2])
            nc.scalar.activation(dst, jsrc, AF.Square, bias=biasc[:, 2:3], scale=-1.0)

        # =========================================================
        # batched GE with virtual partial pivoting
        # =========================================================
        def jslot(k):
            """DVE/ScalarE jastrow ops interleaved into the GE stream."""
            if k == 4:
                nc.vector.tensor_tensor(jacc, jacc, j3, OP.add)
            elif k == 5:
                nc.vector.tensor_tensor(jacc, jacc, j2, OP.add)      # jacc = r2
            elif k == 6:
                nc.scalar.activation(j2, jacc, AF.Ln, bias=biasc[:, 3:4])     # ln r2
                nc.scalar.activation(j3, jacc, AF.Relu,
                                     bias=biasc[:, 5:6], scale=-0.01)    # w - WMIN
                nc.scalar.activation(j5, j2, AF.Exp, scale=-0.5)      # q = 1/r
                nc.scalar.activation(jacc, j2, AF.Exp, scale=0.5)     # r
            elif k == 8:
                nc.vector.tensor_tensor(j1, jacc, fm, OP.mult)        # -r/F
                nc.scalar.activation(j2, j1, AF.Exp)                  # e
                nc.scalar.activation(j1, j3, AF.Ln, bias=biasc[:, 4:5])        # ln w
                nc.scalar.activation(j3, j1, AF.Exp, scale=-1.0)      # 1/w
                nc.scalar.activation(j1, j3, AF.Exp, bias=biasc[:, 6:7], scale=-1.0)  # decay
            elif k == 10:
                # om = 1 - e  (d=32 plane gets half weight)
                nc.scalar.activation(j3[:, 0:ND - 1, :], j2[:, 0:ND - 1, :],
                                     AF.Copy, bias=1.0, scale=-1.0)
                nc.scalar.activation(j3[:, ND - 1, :], j2[:, ND - 1, :],
                                     AF.Copy, bias=0.5, scale=-0.5)
            elif k == 12:
                nc.vector.tensor_tensor(j2, j3, j1, OP.mult)          # om*decay
            elif k == 14:
                nc.vector.tensor_tensor(j1, j2, j5, OP.mult)          # *q
            elif k == 16:
                nc.scalar.activation(j2, j1, AF.Copy, scale=-Aj, accum_out=jsum)

        for k in range(NS):
            T = NS - k
            colk = Abuf[:, :, k, :]
            jslot(k)
            # ---- pivot search ----
            nc.vector.tensor_tensor(c2b, colk, colk, OP.mult)
            nc.vector.tensor_tensor(c2m, c2b, Wt, OP.mult)
            nc.vector.reduce_max(Mb, c2m, axis=AX.X)
            nc.vector.tensor_tensor(
                indb, c2m, Mb[:, :, None].broadcast_to([P, 2, NS]), OP.is_equal
            )
            # ---- pivot row extraction into prow[:, :, k, 0:T] ----
            nc.vector.tensor_tensor(
                scr[:, :, :T, :],
                Abuf[:, :, k:, :],
                indb[:, :, None, :].broadcast_to([P, 2, T, NS]),
                OP.mult,
            )
            nc.vector.reduce_sum(
                _ap(prow, k * NS, [(NS * NS, 2), (1, T)]),
                scr[:, :, :T, :], axis=AX.X,
            )
            if k == NS - 1:
                break
            # ---- multipliers: m = colk / piv, with m[i*] forced to exactly
            # 1.0 so the pivot row self-annihilates to exact zeros (used rows
            # then always lose the max search; no mask bookkeeping needed) ----
            nc.vector.reciprocal(rpv, _ap(prow, k * NS, [(NS * NS, 2), (1, 1)]))
            nc.vector.tensor_tensor(
                mm, colk, rpv.broadcast_to([P, 2, NS]), OP.mult
            )
            nc.vector.copy_predicated(mm, indb, _ap(onec, 0, [(0, 2), (0, NS)]))
            # ---- rank-1 update of trailing columns ----
            nc.vector.tensor_tensor(
                scr[:, :, :T - 1, :],
                _ap(mm, 0, [(NS, 2), (0, T - 1), (1, NS)]),
                _ap(prow, k * NS + 1, [(NS * NS, 2), (1, T - 1), (0, NS)]),
                OP.mult,
            )
            nc.vector.tensor_tensor(
                Abuf[:, :, k + 1:, :].rearrange("p s a b -> p s (a b)"),
                Abuf[:, :, k + 1:, :].rearrange("p s a b -> p s (a b)"),
                scr[:, :, :T - 1, :].rearrange("p s a b -> p s (a b)"),
                OP.subtract,
            )

        # =========================================================
        # logdet tail + combine
        # =========================================================
        pivs = _ap(prow, 0, [(NS * NS, 2), (NS, NS)])   # prow[:, :, k, 0] over k
        labs = pool.tile([P, 2, NS], F32, tag="labs")
        lgb = pool.tile([P, 2, NS], F32, tag="lgb")
        nc.scalar.activation(labs, pivs, AF.Abs)
        nc.scalar.activation(lgb, labs, AF.Ln, bias=biasc[:, 3:4])
        ld2 = pool.tile([P, 2], F32, tag="ld2")
        nc.vector.reduce_sum(ld2, lgb, axis=AX.X)
        ld1 = pool.tile([P, 1], F32, tag="ld1")
        nc.vector.reduce_sum(ld1, ld2, axis=AX.X)
        ob = pool.tile([P, 1], F32, tag="ob")
        nc.vector.tensor_tensor(ob, ld1, jsum, OP.add)
        nc.default_dma_engine.dma_start(outp[:], ob)

    nc.finalize()
    return nc


_CACHE = {}


def _get_built(alpha: float):
    key = round(alpha, 9)
    if key not in _CACHE:
        _CACHE[key] = _build(alpha)
    return _CACHE[key]


def _make_inputs(walkerRs: np.ndarray):
    n = 4
    a = L / n
    coords = np.linspace(0.0, L - a, n).astype(np.float32)
    cen = np.zeros((2, 10), np.float32)
    for s, shift in ((0, 0.0), (1, a / 2)):
        cen[s, 0:2] = coords[:2] + shift
        cen[s, 2:6] = coords[:4] + shift
        cen[s, 6:10] = coords[:4] + shift
    cstv = np.ascontiguousarray(np.broadcast_to(cen[None], (P, 2, 10))).astype(np.float32)

    w = (1.0 + np.arange(NS) * 2.0 ** -20).astype(np.float32)
    wcsv = np.ascontiguousarray(np.broadcast_to(w[None, None, :], (P, 2, NS))).astype(np.float32)

    _, Fs, Fd = _jastrow_consts()
    ii = np.arange(N)
    ddv = np.arange(1, ND + 1)
    same = ((ii[None, :] < NS) == (((ii[None, :] + ddv[:, None]) % N) < NS))
    fmv = np.where(same, -1.0 / Fs, -1.0 / Fd).astype(np.float32)
    fmv = np.ascontiguousarray(np.broadcast_to(fmv[None], (P, ND, N))).astype(np.float32)

    in_maps = []
    for c in range(NCORES):
        sh = walkerRs[c * P:(c + 1) * P]          # (P, N, 3)
        xshv = np.ascontiguousarray(sh.transpose(0, 2, 1)).astype(np.float32)
        in_maps.append({"xsh": xshv, "cst": cstv, "wcs": wcsv, "fmi": fmv})
    return in_maps


def kernel(walkerRs: np.ndarray, log_alpha: np.ndarray, _trace=False):
    walkerRs = np.asarray(walkerRs, dtype=np.float32)
    la = float(np.asarray(log_alpha))
    alpha = float(np.clip(np.exp(la), 55.0 / L ** 2, 300.0 / L ** 2))
    nc = _get_built(alpha)
    in_maps = _make_inputs(walkerRs)
    res = None
    for attempt in range(3):
        try:
            res = run_bass_kernel_spmd(nc, in_maps, list(range(NCORES)),
                                       trace=_trace)
            break
        except Exception:
            # transient NRT "device unrecoverable" after a prior bad run
            if attempt == 2:
                raise
            import time as _time
            _time.sleep(15)
    out = np.concatenate([res.results[i]["out"][:, 0] for i in range(NCORES)])
    if _trace:
        return out.astype(np.float32), res
    return out.astype(np.float32)
